# revision 67
# baseline (speedup 1.0000x reference)
"""BitNet attention (GQA, 32 q-heads / 8 kv-heads, hidden 4096, seq 2048) on 8
Trainium2 NeuronCores.

Sharding: tensor-parallel over heads. Core i computes q-heads 4i..4i+3 and
kv-head i (N_REP=4, so the 4 q-heads of core i attend exactly to kv-head i),
plus the o_proj contribution of its 512 hidden columns; the host sums the 8
partial o_proj outputs.

Device-side layout choices (per core):
  - Q/K/V projections run in fp8e4m3 with perf_mode=DoubleRow (2 contraction
    chunks per matmul, ~1.7x bf16 rate). x ships as an fp8 hi/lo pair
    (lo = e4m3(16*(x-hi))): Q/K contract hi-only (softmax damps the ~2%
    activation quantization noise); V contracts (hi, lo) against duplicated
    weight planes (sign(W), sign(W)/16), recovering ~bf16 accuracy on the
    value path, which feeds the output directly and cannot tolerate fp8
    noise. Scores/AV/o_proj stay bf16 (contraction 128 can't DoubleRow;
    o_proj input is accuracy-critical). PSUM accumulation is fp32.
  - x is passed transposed as xT; all streaming blocks are repacked on the
    host into partition-major contiguous layouts so each block is ONE DMA
    (descriptor-issue on the sync engine was a startup bottleneck).
  - Q/K are produced feature-on-partition (Qt/Kt = [d, T]); V token-on-
    partition ([T, d]), with the per-d v-scale folded into the V copyout
    (ACT scale vector), so softmax normalization is a pure 1/E multiply.
  - Scores are computed transposed, S.T[k, q] = Kt_tile^T @ Qt, so the exp
    output P.T[k, q] feeds directly as lhsT/rhs of the O.T matmuls without
    any transposes. Scores PSUM tiles span 2 banks (two key tiles) so each
    ACT exp op amortizes its ~352-cycle fixed overhead.
  - softmax has no max-subtraction (scores are O(1) by construction: binary
    weights with per-row mean-abs rescale keep |scores| ~ 1).
  - The softmax denominator E[q] is a GpSimd partition-reduce of the
    DVE-pre-summed P.T tiles (GpSimd is otherwise idle); 1/E is computed on
    DVE via a [1,512]->[128,4] reshape DMA, and broadcast back to 128
    partitions with a GpSimd partition_broadcast. ot = O.T * (1/E) is one
    DVE multiply. This keeps the PE free of the 32 small E/sc matmuls the
    previous version used, which each ran at ~375ns and stalled the PE
    once per iteration.
  - o_proj is software-pipelined INTO the attention loop: once query-block
    qb's four heads are finalized (~iteration 4*qb+5), its 16 o_proj PSUM
    tiles are emitted 4-6 per iteration. This hides the ACT engine's exp
    surplus (8 exps/iter = 8.3us > the 6.9us of scores+AV matmuls) behind
    o_proj PE work, and spreads the y output DMA across the whole second
    half of the kernel instead of a back-loaded phase 3.
  - PE-engine self-waits (waits on the PE's own tick semaphore) are elided
    post-scheduling: PE matmuls complete in queue order and only write
    PSUM, so those waits are implied -- and removing them restores the
    LDWEIGHTS pull-ahead that standalone wait instructions block (~135
    matmuls ran at ~355-379ns instead of ~214ns because of this).
"""

import re

import numpy as np
import ml_dtypes

import concourse.bass as bass
import concourse.mybir as mybir
import concourse.tile as tile
from concourse.vector_clock import ScopedClock
from concourse.bass_utils import run_bass_kernel_spmd

F32 = mybir.dt.float32
BF16 = mybir.dt.bfloat16
F8 = mybir.dt.float8e4  # e4m3, DoubleRow-capable
DR = mybir.MatmulPerfMode.DoubleRow

HIDDEN = 4096
T = 2048          # sequence length
N_CORES = 8
FQ = HIDDEN // N_CORES   # 512 q-features per core
H = 4                    # q heads per core
DH = 128                 # head dim
DC = HIDDEN // 128       # 32 contraction chunks
HC = DC // 2             # 16 chunks per xt half
TQ = 4                   # token quarters (512 tokens each)
KT = T // 128            # 16 key tiles
QB = 4                   # query blocks of 512

TRACE = False            # set by test.py for profiling runs
TRACE_ALL_CORES = False

_MAX_DRAIN_WAITS = 1
_MAX_INST_WAITS = 1

_PE_TICK_SEM = re.compile(r"^PE_\d+$")


def _split_sync_waits(nc):
    """Two post-scheduling wait fixups:

    1. Elide PE self-waits: a wait on the PE's own tick semaphore by a PE
       instruction is implied by queue order (matmuls complete pc-ordered
       and only write PSUM; ldweights writes only the weight regs), and a
       standalone wait instruction on the PE queue blocks the hardware's
       LDWEIGHTS pull-ahead, serializing the weight load with the previous
       matmul (~+140ns each).

    2. The walrus build in this container rejects instructions carrying more
       than one sync wait ("Too many sync wait commands"). Cap every
       instruction at _MAX_INST_WAITS waits; spill the excess onto
       InstEventSemaphore (standalone wait) instructions inserted immediately
       before on the same engine (engines are in-order, so combined wait
       semantics are identical)."""
    counter = [0]

    def _mk_wait(engine, waits):
        counter[0] += 1
        nop = mybir.InstEventSemaphore(
            name=f"waitsplit_{counter[0]}", ins=[], outs=[]
        )
        nop.engine = engine
        nop.sync_info = mybir.SyncInfo(on_wait=list(waits), on_update=[])
        nc.register_instruction(nop, overwrite=True)
        return nop

    for bb in nc.main_func.blocks:
        insts = list(bb.instructions)
        out = []
        changed = False
        for ins in insts:
            si = ins.sync_info
            waits = list(si.on_wait or []) if si else []
            if waits and ins.engine == mybir.EngineType.PE:
                kept = [
                    w for w in waits
                    if not (
                        getattr(w, "ant_name", None)
                        and _PE_TICK_SEM.match(w.ant_name)
                        and getattr(w, "wait_mode", None) == "sem-ge-imm"
                    )
                ]
                if len(kept) != len(waits):
                    ins.sync_info = mybir.SyncInfo(
                        on_wait=kept, on_update=list(si.on_update or [])
                    )
                    waits = kept
                    changed = True
            if len(waits) > _MAX_INST_WAITS:
                changed = True
                rest = waits[:-_MAX_INST_WAITS]
                for i in range(0, len(rest), _MAX_INST_WAITS):
                    out.append(_mk_wait(ins.engine, rest[i : i + _MAX_INST_WAITS]))
                ins.sync_info = mybir.SyncInfo(
                    on_wait=waits[-_MAX_INST_WAITS:],
                    on_update=list(si.on_update or []),
                )
            out.append(ins)
        if changed:
            bb.instructions = out


class _PatchedTileContext(tile.TileContext):
    """Split the end-of-kernel drain's sem waits the same way (the drain is
    emitted after scheduling, outside _split_sync_waits' reach)."""

    def _drain_and_barrier(self, tick_clock, wait_clock):
        nc = self.nc
        drain_inst = nc.sync.drain()
        wait_clock.add_sem_waits(
            drain_inst.ins, ScopedClock({None: tick_clock.global_clock})
        )
        ins = drain_inst.ins
        si = ins.sync_info
        waits = list(si.on_wait or []) if si else []
        updates = list(si.on_update or []) if si else []
        if len(waits) > _MAX_DRAIN_WAITS:
            ins.sync_info = mybir.SyncInfo(
                on_wait=waits[:_MAX_DRAIN_WAITS], on_update=updates
            )
            rest = waits[_MAX_DRAIN_WAITS:]
            for i in range(0, len(rest), _MAX_DRAIN_WAITS):
                nop = nc.sync.nop(nofuse=True, hint=f"dw{i}")
                nop.ins.sync_info = mybir.SyncInfo(
                    on_wait=rest[i : i + _MAX_DRAIN_WAITS], on_update=[]
                )
        nc.all_engine_barrier()
        assert self.sems is not None
        popped = nc._tile_sem_poison_stack.pop()
        assert popped is self._sem_poison
        nc.clear_and_free_semaphores(list(self.sems.allocated().values()))
        nc.all_engine_barrier()


def _build(split_waits=True):
    nc = bass.Bass()

    # partition-major packed inputs (see _make_in_maps)
    # xt: fp8 hi/lo split of x.T — plane 0 = e4m3(x), plane 1 = e4m3(16*(x-hi)).
    # Q/K contract hi-only via DoubleRow over dc pairs; V contracts (hi, lo)
    # pairs against (sign(W), sign(W)/16) weight planes, recovering ~bf16
    # accuracy for the V path.
    xt_d = nc.dram_tensor("xt", [TQ, 2, 2, 128, HC, 512], F8, kind="ExternalInput")
    bqt_d = nc.dram_tensor("bqt", [128, DC, FQ], F8, kind="ExternalInput")
    bkt_d = nc.dram_tensor("bkt", [128, DC, DH], F8, kind="ExternalInput")
    bvt_d = nc.dram_tensor("bvt", [128, DC, 2, DH], F8, kind="ExternalInput")
    bot_d = nc.dram_tensor("bot", [4, 128, H, 1024], BF16, kind="ExternalInput")
    # packed constants: one descriptor each (descriptor-gen on the Sync
    # engine is ~624ns per dma_start and gates startup)
    cst_d = nc.dram_tensor("cst", [DH, 8], F32, kind="ExternalInput")
    # identx: cols 0..127 = identity (transposes), col 128 = ones (E row-sum)
    identx_d = nc.dram_tensor("identx", [128, 129], BF16, kind="ExternalInput")
    sv_d = nc.dram_tensor("sv", [1, DH], BF16, kind="ExternalInput")
    y_d = nc.dram_tensor("y", [T, HIDDEN], BF16, kind="ExternalOutput")

    from contextlib import ExitStack
    with _PatchedTileContext(nc) as tc, ExitStack() as _ctx:
        wq = _ctx.enter_context(tc.tile_pool(name="wq", bufs=1))
        wk = _ctx.enter_context(tc.tile_pool(name="wk", bufs=1))
        wv = _ctx.enter_context(tc.tile_pool(name="wv", bufs=1))
        xtp = _ctx.enter_context(tc.tile_pool(name="xt", bufs=3))
        qtp = _ctx.enter_context(tc.tile_pool(name="qt", bufs=H))
        ktp = _ctx.enter_context(tc.tile_pool(name="kt", bufs=1))
        vvp = _ctx.enter_context(tc.tile_pool(name="vv", bufs=TQ))
        ptp = _ctx.enter_context(tc.tile_pool(name="pt", bufs=16))
        otp = _ctx.enter_context(tc.tile_pool(name="ot", bufs=H))
        wop = _ctx.enter_context(tc.tile_pool(name="wo", bufs=4))
        ysp = _ctx.enter_context(tc.tile_pool(name="ys", bufs=4))
        vtp = _ctx.enter_context(tc.tile_pool(name="vt", bufs=2))
        scp = _ctx.enter_context(tc.tile_pool(name="sc", bufs=2))
        misc = _ctx.enter_context(tc.tile_pool(name="misc", bufs=2))
        psM = _ctx.enter_context(tc.tile_pool(name="psM", bufs=2, space="PSUM"))
        psS = _ctx.enter_context(tc.tile_pool(name="psS", bufs=3, space="PSUM"))
        if True:
            # --- xt first pieces FIRST so compute starts ASAP -------------
            xt_sb = {}  # (tq, half) -> [128, 2, HC, 512] fp8 tile (hi|lo)

            def load_xt(tq, half):
                # hi plane in quarters, lo in halves: consumers wait on the
                # piece, not the whole 2MB tile, so the PE can chase the
                # stream during the bandwidth-bound startup window
                t_ = xtp.tile([128, 2, HC, 512], F8, tag="xt",
                              name=f"xt{tq}_{half}")
                q4 = HC // 4
                for qtr in range(4):
                    sl = slice(qtr * q4, (qtr + 1) * q4)
                    nc.sync.dma_start(
                        t_[:, 0, sl, :], xt_d[tq, half, 0, :, sl, :]
                    )
                for hlf in range(2):
                    sl = slice(hlf * (HC // 2), (hlf + 1) * (HC // 2))
                    nc.sync.dma_start(
                        t_[:, 1, sl, :], xt_d[tq, half, 1, :, sl, :]
                    )
                xt_sb[(tq, half)] = t_

            def xt_hi_pair(tq, dp):
                # [128, 2, 512] hi planes of chunks (2dp, 2dp+1)
                dc = 2 * dp
                return xt_sb[(tq, dc // HC)][:, 0, dc % HC : dc % HC + 2, :]

            def xt_hilo(tq, dc):
                # [128, 2, 512] (hi, lo) planes of chunk dc
                return xt_sb[(tq, dc // HC)][:, :, dc % HC, :]

            bqt_sb = wq.tile([128, DC, FQ], F8, tag="wq")
            bkt_sb = wk.tile([128, DC, DH], F8, tag="wk")
            bvt_sb = wv.tile([128, DC, 2, DH], F8, tag="wv")

            # The very first bite: 2 weight chunks + 2 x chunks (128KB each)
            # so MM0 fires as early as the descriptor queue allows. Each
            # DIRECT2D descriptor costs ~624ns on the Sync engine, so the
            # order of these first few dma_starts IS the startup critical
            # path.
            nc.sync.dma_start(bqt_sb[:, 0:2], bqt_d[:, 0:2])
            xt_sb[(0, 0)] = xtp.tile([128, 2, HC, 512], F8, tag="xt",
                                     name="xt0_0")
            nc.sync.dma_start(
                xt_sb[(0, 0)][:, 0, 0:2, :], xt_d[0, 0, 0, :, 0:2, :]
            )
            nc.sync.dma_start(bqt_sb[:, 2:4], bqt_d[:, 2:4])
            nc.sync.dma_start(
                xt_sb[(0, 0)][:, 0, 2:4, :], xt_d[0, 0, 0, :, 2:4, :]
            )
            # full bkt early, one descriptor (0.5MB): tq0 is consumed
            # chunk-major by TWO matmul streams (Q head 0 + K) so the PE's
            # first-touch rate stays under the DMA delivery rate
            nc.sync.dma_start(bkt_sb[:], bkt_d[:])
            # packed constants (3 descriptors; the Q0 copyout on ACT needs
            # sq ~7us after MM0)
            cst_sb = misc.tile([DH, 8], F32, tag="cst")
            nc.sync.dma_start(cst_sb[:], cst_d[:])
            identx_sb = misc.tile([128, 129], BF16, tag="identx")
            nc.sync.dma_start(identx_sb[:], identx_d[:])
            sv_sb = misc.tile([1, DH], BF16, tag="sv")
            nc.sync.dma_start(sv_sb[:], sv_d[:])
            sq_col = [cst_sb[:, f : f + 1] for f in range(H)]
            sk_col = cst_sb[:, 4:5]
            ident_ap = identx_sb[:, 0:128]
            ones_ap = identx_sb[:, 128:129]
            # fine-grained interleave so Q matmul dp (needs bqt chunks
            # 2dp,2dp+1 + xt chunks 2dp,2dp+1) can start as early as possible
            pieces = [(4, 8), (8, 12), (12, 16)]
            pieces_h1 = [(2, 4), (4, 8), (8, 12), (12, 16)]
            for half_ in range(2):
                if half_ == 1:
                    pieces = pieces_h1
                    xt_sb[(0, 1)] = xtp.tile(
                        [128, 2, HC, 512], F8, tag="xt", name="xt0_1"
                    )
                    nc.sync.dma_start(
                        bqt_sb[:, HC : HC + 2], bqt_d[:, HC : HC + 2]
                    )
                    nc.sync.dma_start(
                        xt_sb[(0, 1)][:, 0, 0:2, :], xt_d[0, 1, 0, :, 0:2, :]
                    )
                for a, b in pieces:
                    nc.sync.dma_start(
                        bqt_sb[:, half_ * HC + a : half_ * HC + b],
                        bqt_d[:, half_ * HC + a : half_ * HC + b],
                    )
                    nc.sync.dma_start(
                        xt_sb[(0, half_)][:, 0, a:b, :],
                        xt_d[0, half_, 0, :, a:b, :],
                    )
            for qc in range(4):
                sl = slice(qc * (DC // 4), (qc + 1) * (DC // 4))
                nc.sync.dma_start(bvt_sb[:, sl], bvt_d[:, sl])
            for half_ in range(2):
                for hlf in range(2):
                    sl = slice(hlf * (HC // 2), (hlf + 1) * (HC // 2))
                    nc.sync.dma_start(
                        xt_sb[(0, half_)][:, 1, sl, :],
                        xt_d[0, half_, 1, :, sl, :],
                    )

            # --- persistent activation tiles ----------------------------
            qt_sb = [qtp.tile([DH, T], BF16, tag="qt", name=f"qt{f}")
                     for f in range(H)]
            kt_sb = ktp.tile([DH, T], BF16, tag="kt")
            vv_sb = [vvp.tile([128, 512], BF16, tag="vv", name=f"vv{tq}")
                     for tq in range(TQ)]
            ot_sb = [otp.tile([DH, T], BF16, tag="ot", name=f"ot{f}")
                     for f in range(H)]

            # --- phase 1: q/k/v projections, one token-quarter at a time
            def emit_q(tq, f):
                tsl = slice(tq * 512, (tq + 1) * 512)
                ps = psM.tile([128, 512], F32, tag="mm", name=f"psq{tq}_{f}")
                for dp in range(DC // 2):
                    nc.tensor.matmul(
                        ps[:],
                        bqt_sb[:, 2 * dp : 2 * dp + 2, f * 128 : (f + 1) * 128],
                        xt_hi_pair(tq, dp),
                        start=(dp == 0), stop=(dp == DC // 2 - 1),
                        perf_mode=DR,
                    )
                nc.scalar.activation(
                    qt_sb[f][:, tsl], ps[:],
                    mybir.ActivationFunctionType.Copy, scale=sq_col[f],
                )

            def emit_k(tq):
                tsl = slice(tq * 512, (tq + 1) * 512)
                ps = psM.tile([128, 512], F32, tag="mm", name=f"psk{tq}")
                for dp in range(DC // 2):
                    nc.tensor.matmul(
                        ps[:], bkt_sb[:, 2 * dp : 2 * dp + 2, :],
                        xt_hi_pair(tq, dp),
                        start=(dp == 0), stop=(dp == DC // 2 - 1),
                        perf_mode=DR,
                    )
                nc.scalar.activation(
                    kt_sb[:, tsl], ps[:],
                    mybir.ActivationFunctionType.Copy, scale=sk_col,
                )

            def emit_v_fin(tq, ps):
                # psum -> bf16, then 4 PE transposes back to [t, d]
                vt_sb = vtp.tile([128, 512], BF16, tag="vt", name=f"vt{tq}")
                nc.vector.tensor_copy(out=vt_sb[:], in_=ps[:])
                for vt in range(4):
                    ps_tr = psS.tile([128, 128], BF16, tag="s2",
                                     name=f"pstr{tq}_{vt}")
                    nc.tensor.transpose(
                        ps_tr[:], vt_sb[:, vt * 128 : (vt + 1) * 128],
                        ident_ap,
                    )
                    nc.vector.tensor_copy(
                        out=vv_sb[tq][:, vt * 128 : (vt + 1) * 128],
                        in_=ps_tr[:],
                    )

            def emit_v(tq):
                # V.T (weights stationary, full-width moving operand).
                # Contracts (hi, lo) fp8 pairs against (sign(W), sign(W)/16)
                # for ~bf16 accuracy.
                ps = psM.tile([128, 512], F32, tag="mm", name=f"psv{tq}")
                for dc in range(DC):
                    nc.tensor.matmul(
                        ps[:], bvt_sb[:, dc], xt_hilo(tq, dc),
                        start=(dc == 0), stop=(dc == DC - 1),
                        perf_mode=DR,
                    )
                emit_v_fin(tq, ps)

            def emit_score_pair(h, qb, kp, pt_list):
                qsl = slice(qb * 512, (qb + 1) * 512)
                ps_s = psS.tile([128, 1024], F32, tag="s2",
                                name=f"pss{h}_{qb}_{kp}")
                for j in range(2):
                    kt = 2 * kp + j
                    nc.tensor.matmul(
                        ps_s[:, j * 512 : (j + 1) * 512],
                        kt_sb[:, kt * 128 : (kt + 1) * 128],
                        qt_sb[h][:, qsl],
                        start=True, stop=True,
                    )
                pt = ptp.tile([128, 1024], BF16, tag="pt",
                              name=f"pt{h}_{qb}_{kp}")
                nc.scalar.activation(
                    pt[:], ps_s[:], mybir.ActivationFunctionType.Exp
                )
                pt_list.append(pt)

            bot_sb = {}

            def load_bot(obp):
                t_ = wop.tile([128, H, 1024], BF16, tag="wo", name=f"wo{obp}")
                nc.sync.dma_start(t_[:], bot_d[obp])
                bot_sb[obp] = t_

            pro_pt = []  # iteration (h0, qb0) scores, emitted into tq3
            for tq in range(TQ):
                if tq > 0:
                    load_xt(tq, 0)
                    load_xt(tq, 1)
                    # o-weights DMA'd during phase 1: issuing them in the
                    # attention loop put the tiny reciprocal reshape DMAs
                    # behind these 1MB transfers (~2.5us stalls at the sc
                    # matmul in bodies 2-5)
                    load_bot(tq - 1)
                    if tq == 3:
                        load_bot(3)
                if tq < 3:
                    for f in range(H):
                        emit_q(tq, f)
                    emit_k(tq)
                    emit_v(tq)
                else:
                    # interleave iteration-0 scores into the projection tail:
                    # Kt completes after emit_k(3); Qt[0] after emit_q(3, 0).
                    # emit_v(3) is DEFERRED into body 1 of the attention
                    # loop (spread over kpp 0-2, done before AV kpp3 needs
                    # vv3) -- it pads the ACT-bound first body with PE work.
                    emit_q(3, 0)
                    emit_k(3)
                    emit_q(3, 1)
                    for kp in range(0, 2):
                        emit_score_pair(0, 0, kp, pro_pt)
                    emit_q(3, 2)
                    for kp in range(2, 5):
                        emit_score_pair(0, 0, kp, pro_pt)
                    emit_q(3, 3)
                    for kp in range(5, 8):
                        emit_score_pair(0, 0, kp, pro_pt)

            # --- phase 2: attention + o_proj, software-pipelined ---------
            # Iteration J = (h, qb). Body(idx) interleaves, at score-pair
            # granularity, scores+exp of iters[idx] with the O.T matmuls of
            # iters[idx-1]. The softmax-denominator pipeline for iter j is
            # spread over three bodies so the PE never waits on the DVE:
            #   body j+1 end:   DVE pair-sum tree -> etree tile
            #   body j+2 kpp1:  E row-sum matmul (ones^T @ etree) + the
            #                   reshape-DMA reciprocal chain on DVE
            #   body j+3 kpp0:  sc = sv (x) 1/E outer-product matmul +
            #                   ot = O.T * sc on DVE
            # (the previous version issued the E matmul at body j+1 end,
            # where it stalled the PE ~0.5us/iter waiting for the tree).
            #
            # o_proj tiles for query block qb become emittable at body
            # 4*qb+6 kpp1 (after the (3,qb) finalize at kpp0) and are
            # spread over the following bodies; query block 3's 16 tiles
            # drain after the loop.
            iters = [(h, qb) for qb in range(QB) for h in range(H)]
            pend_tree = []   # (h, qb, etree, ps_o): E matmul due next body
            pend_fin = []    # (h, qb, ps_o, recip_bf): sc+mult due body after
            prev = (0, 0, pro_pt, None)

            def emit_o_tile(obp, tt, use_act=False):
                osl = slice(obp * 1024, (obp + 1) * 1024)
                ps_y = psS.tile([128, 1024], F32, tag="s2",
                                name=f"psy{obp}_{tt}")
                for jo in range(2):
                    for c in range(H):
                        nc.tensor.matmul(
                            ps_y[:, jo * 512 : (jo + 1) * 512],
                            ot_sb[c][:, tt * 128 : (tt + 1) * 128],
                            bot_sb[obp][:, c, jo * 512 : (jo + 1) * 512],
                            start=(c == 0), stop=(c == H - 1),
                        )
                ysb = ysp.tile([128, 1024], BF16, tag="ys",
                               name=f"ys{obp}_{tt}")
                # steady-state y copyouts on DVE (ACT must keep pace with
                # the exp stream, which sets the scores-PSUM recycle rate);
                # drain-region copyouts on ACT (idle there, DVE busy with
                # the final reciprocal chain)
                if use_act:
                    nc.scalar.activation(
                        ysb[:], ps_y[:], mybir.ActivationFunctionType.Copy
                    )
                else:
                    nc.vector.tensor_copy(out=ysb[:], in_=ps_y[:])
                nc.sync.dma_start(
                    y_d[tt * 128 : (tt + 1) * 128, osl], ysb[:]
                )

            # o_proj tile schedule: qb k's 16 tiles -> bodies 4k+6..4k+9
            # (qb 2 squeezed into 14/15/16, qb 3 after the loop).
            o_assign = {i: [] for i in range(1, len(iters) + 1)}
            o_post = []
            for k in range(QB):
                tiles = [(obp, tt) for obp in range(4)
                         for tt in range(4 * k, 4 * k + 4)]
                if k < 2:
                    for j, t_ in enumerate(tiles):
                        o_assign[4 * k + 6 + j // 4].append(t_)
                elif k == 2:
                    # hold 2 tiles back for the post-loop seam: they don't
                    # need iter 15's finalize and keep the PE busy while the
                    # last reciprocal chain completes
                    o_assign[14].extend(tiles[:5])
                    o_assign[15].extend(tiles[5:10])
                    o_assign[16].extend(tiles[10:13])
                    o_seam = tiles[13:]
                else:
                    o_post.extend(tiles)

            def emit_e_chain(st):
                # E directly in [128, 4] layout: matmul j contracts etree's
                # strided column slice {j, j+4, j+8, ...} with ones, so
                # ps_et[p, j] = E[4p + j]. This skips the [1,512]->[128,4]
                # reshape DMA (which cost ~1.3-2.5us of chain latency), and
                # the DVE reciprocal reads PSUM and writes bf16 directly.
                h, qb, etree, ps_o = st
                ps_et = psS.tile([128, 4], F32, tag="s2", name=f"pse{h}_{qb}")
                for j in range(4):
                    nc.tensor.matmul(
                        ps_et[:, j : j + 1],
                        etree[:, j::4],
                        ones_ap,
                        start=True, stop=True,
                    )
                recip_bf128 = misc.tile([128, 4], BF16, tag="recipbf128",
                                        name=f"recipbf128{h}_{qb}")
                with nc.allow_low_precision(
                    reason="1/E stored bf16 by design (matches prior "
                    "f32-recip + bf16-cast rounding)"
                ):
                    nc.vector.reciprocal(recip_bf128[:], ps_et[:])
                recip_bf = misc.tile([1, 512], BF16, tag="recipbf",
                                     name=f"recipbf{h}_{qb}")
                nc.sync.dma_start(recip_bf[:], recip_bf128[:])
                pend_fin.append((h, qb, ps_o, recip_bf))

            def finalize2(st):
                h, qb, ps_o, recip = st
                qsl = slice(qb * 512, (qb + 1) * 512)
                ps_sc = psS.tile([128, 512], F32, tag="s2",
                                 name=f"pssc{h}_{qb}")
                nc.tensor.matmul(
                    ps_sc[:], sv_sb[:], recip[:], start=True, stop=True
                )
                sc_sb = scp.tile([128, 512], F32, tag="sc", name=f"sc{h}_{qb}")
                nc.vector.tensor_copy(out=sc_sb[:], in_=ps_sc[:])
                nc.vector.tensor_tensor(
                    ot_sb[h][:, qsl], ps_o[:], sc_sb[:], mybir.AluOpType.mult
                )

            # deferred tq3 V projection: 32 DR matmuls spread over body 1's
            # kpp 0-2 (plus copy/transposes at kpp2), finishing just before
            # AV kpp3 reads vv3. Pads the ACT-bound first body with PE work.
            ps_v3 = [None]
            v3_ranges = [(0, 11), (11, 22), (22, 32)]

            def emit_v3_part(kpp):
                a, b = v3_ranges[kpp]
                if kpp == 0:
                    ps_v3[0] = psM.tile([128, 512], F32, tag="mm",
                                        name="psv3")
                for dc in range(a, b):
                    nc.tensor.matmul(
                        ps_v3[0][:], bvt_sb[:, dc], xt_hilo(3, dc),
                        start=(dc == 0), stop=(dc == DC - 1),
                        perf_mode=DR,
                    )
                if kpp == 2:
                    emit_v_fin(3, ps_v3[0])

            for idx in range(1, len(iters) + 1):
                cur = iters[idx] if idx < len(iters) else None
                if idx <= 4:
                    load_bot(idx - 1)
                otiles = o_assign[idx]
                nper = (len(otiles) + 2) // 3 if otiles else 0
                new_pt = []
                if prev is not None:
                    ph, pqb, ppt, _ = prev
                    ps_o = psM.tile([128, 512], F32, tag="mm",
                                    name=f"pso{ph}_{pqb}")
                    prev = (ph, pqb, ppt, ps_o)
                for kpp in range(KT // 4):
                    for kp in (2 * kpp, 2 * kpp + 1):
                        if cur is not None:
                            h, qb = cur
                            qsl = slice(qb * 512, (qb + 1) * 512)
                            ps_s = psS.tile([128, 1024], F32, tag="s2",
                                            name=f"pss{h}_{qb}_{kp}")
                            for j in range(2):
                                kt = 2 * kp + j
                                nc.tensor.matmul(
                                    ps_s[:, j * 512 : (j + 1) * 512],
                                    kt_sb[:, kt * 128 : (kt + 1) * 128],
                                    qt_sb[h][:, qsl],
                                    start=True, stop=True,
                                )
                            pt = ptp.tile([128, 1024], BF16, tag="pt",
                                          name=f"pt{h}_{qb}_{kp}")
                            nc.scalar.activation(
                                pt[:], ps_s[:],
                                mybir.ActivationFunctionType.Exp,
                            )
                            new_pt.append(pt)
                    if prev is not None:
                        ph, pqb, ppt, ps_o = prev
                        for kt in range(4 * kpp, 4 * kpp + 4):
                            tqi, vti = divmod(kt, 4)
                            rhs = ppt[kt // 2][:, (kt % 2) * 512
                                               : (kt % 2 + 1) * 512]
                            nc.tensor.matmul(
                                ps_o[:],
                                vv_sb[tqi][:, vti * 128 : (vti + 1) * 128],
                                rhs,
                                start=(kt == 0), stop=(kt == KT - 1),
                            )
                        # pre-sum the exp pair on the (idle) DVE so the
                        # GpSimd partition-reduce sees a single tile
                        nc.vector.tensor_tensor(
                            ppt[2 * kpp][:], ppt[2 * kpp][:],
                            ppt[2 * kpp + 1][:], mybir.AluOpType.add,
                        )
                        if kpp % 2 == 1:
                            nc.vector.tensor_tensor(
                                ppt[2 * kpp - 2][:], ppt[2 * kpp - 2][:],
                                ppt[2 * kpp][:], mybir.AluOpType.add,
                            )
                        if kpp == 3 and cur is None:
                            # last body: fold + start the E chain NOW, ahead
                            # of the otile y-copies queued on DVE, so the
                            # final reciprocal is ready at the drain seam
                            nc.vector.tensor_tensor(
                                ppt[0][:], ppt[0][:], ppt[4][:],
                                mybir.AluOpType.add,
                            )
                            etree = scp.tile([128, 512], BF16, tag="etr",
                                             name=f"etr{ph}_{pqb}")
                            nc.vector.tensor_tensor(
                                etree[:], ppt[0][:, :512], ppt[0][:, 512:],
                                mybir.AluOpType.add,
                            )
                            emit_e_chain((ph, pqb, etree, ps_o))
                            prev = None
                    if idx == 1 and kpp <= 2:
                        emit_v3_part(kpp)
                    if kpp == 1 and pend_tree:
                        emit_e_chain(pend_tree.pop(0))
                    if kpp == 3 and pend_fin:
                        finalize2(pend_fin.pop(0))
                    if kpp >= 1:
                        for t_ in otiles[(kpp - 1) * nper : kpp * nper]:
                            emit_o_tile(*t_, use_act=(cur is None))
                if prev is not None:
                    ph, pqb, ppt, ps_o = prev
                    # fold the remaining tree levels on DVE into a dedicated
                    # tile (keeps pt buffer lifetimes within two bodies)
                    nc.vector.tensor_tensor(
                        ppt[0][:], ppt[0][:], ppt[4][:], mybir.AluOpType.add
                    )
                    etree = scp.tile([128, 512], BF16, tag="etr",
                                     name=f"etr{ph}_{pqb}")
                    nc.vector.tensor_tensor(
                        etree[:], ppt[0][:, :512], ppt[0][:, 512:],
                        mybir.AluOpType.add,
                    )
                    pend_tree.append((ph, pqb, etree, ps_o))
                prev = (cur[0], cur[1], new_pt, None) if cur else None

            # --- drain: iter 15's E chain was issued inside body 16; run
            # the seam tiles (qb2 work that does not need fin(15)) while
            # its reciprocal completes, then fin(15), then qb 3's o_proj.
            # All drain copyouts ride the (idle-here) ACT engine.
            for obp, tt in o_seam:
                emit_o_tile(obp, tt, use_act=True)
            while pend_fin:
                finalize2(pend_fin.pop(0))
            for obp, tt in o_post:
                emit_o_tile(obp, tt, use_act=True)

    if split_waits:
        _split_sync_waits(nc)
    return nc


_NC_CACHE = None


def _get_nc():
    global _NC_CACHE
    if _NC_CACHE is None:
        _NC_CACHE = _build()
    return _NC_CACHE


def _binarize(w):
    """Match reference bitnet_linear: s = max(mean|W|_row, 1e-8) (>0), so
    sign(W/s) == sign(W). Returns (sign(W) as bf16, s as f32)."""
    w = np.asarray(w, np.float32)
    s = np.maximum(
        np.abs(w).mean(axis=1, dtype=np.float64).astype(np.float32), 1e-8
    )
    return np.sign(w).astype(ml_dtypes.bfloat16), s


def _make_in_maps(hidden_states, q_weight, q_scale, k_weight, k_scale,
                  v_weight, v_scale, o_weight, o_scale):
    hs = np.asarray(hidden_states, np.float32)
    b, t, hid = hs.shape
    assert (b, t, hid) == (1, T, HIDDEN)

    xT = np.ascontiguousarray(hs[0].T)  # [d, t] fp32
    # fp8 hi/lo split: x ~= hi + lo/16 with both planes e4m3
    E4 = ml_dtypes.float8_e4m3
    xhi = xT.astype(E4)
    xlo = ((xT - xhi.astype(np.float32)) * 16.0).astype(E4)
    # [d, t] -> [tq, half, p, c_in_half, hl, f]  (d = (half*HC + c)*128 + p,
    #                                             t = tq*512 + f)

    def packx(a):
        return a.reshape(2, HC, 128, TQ, 512).transpose(3, 0, 2, 1, 4)

    xt4 = np.ascontiguousarray(
        np.stack([packx(xhi), packx(xlo)], axis=2)
    )  # [TQ, 2, 2, 128, HC, 512] fp8 (hi plane then lo plane, contiguous)

    bq, s_q = _binarize(q_weight)
    bk, s_k = _binarize(k_weight)
    bv, s_v = _binarize(v_weight)
    bo, s_o = _binarize(o_weight)

    sq_full = s_q * np.asarray(q_scale, np.float32)                # [4096]
    sk_full = s_k * np.asarray(k_scale, np.float32) / np.sqrt(DH)  # [1024]
    sv_full = s_v * np.asarray(v_scale, np.float32)                # [1024]
    so_full = s_o * np.asarray(o_scale, np.float32)                # [4096]

    # identx: identity (for PE transposes) with a ones column appended
    # (E row-sum lhsT) -- one DMA descriptor instead of two
    identx = np.ones((128, 129), dtype=ml_dtypes.bfloat16)
    identx[:, :128] = np.eye(128, dtype=ml_dtypes.bfloat16)

    def _pack_cst(sq8, sk8):
        # [DH, 8] f32: cols 0-3 per-head q scales, col 4 k scale
        cst = np.zeros((DH, 8), np.float32)
        cst[:, 0:4] = sq8.reshape(H, DH).T
        cst[:, 4] = sk8
        return np.ascontiguousarray(cst)

    def pack_w(wt, nf):
        # [d, nf] -> [p, c, nf]
        return np.ascontiguousarray(wt.reshape(DC, 128, nf).transpose(1, 0, 2))

    def pack_w8(wt, nf):
        return pack_w(wt, nf).astype(E4)

    def pack_v8(wt):
        # [d, DH] -> [p, c, 2, DH] with planes (sign(W), sign(W)/16)
        pw = pack_w(wt.astype(np.float32), DH)
        return np.ascontiguousarray(
            np.stack([pw, pw / 16.0], axis=2).astype(E4)
        )

    in_maps = []
    for i in range(N_CORES):
        fq = slice(FQ * i, FQ * (i + 1))
        fk = slice(DH * i, DH * (i + 1))
        bot = np.ascontiguousarray(bo[:, fq].T)  # [512 cfeat, 4096 o]
        in_maps.append({
            "xt": xt4,
            "bqt": pack_w8(np.ascontiguousarray(bq[fq].T), FQ),
            "bkt": pack_w8(np.ascontiguousarray(bk[fk].T), DH),
            "bvt": pack_v8(np.ascontiguousarray(bv[fk].T)),
            "bot": np.ascontiguousarray(
                bot.reshape(H, 128, 4, 1024).transpose(2, 1, 0, 3)
            ),
            "cst": _pack_cst(sq_full[fq], sk_full[fk]),
            "sv": np.ascontiguousarray(
                sv_full[fk].reshape(1, DH).astype(ml_dtypes.bfloat16)
            ),
            "identx": identx,
        })
    return in_maps, so_full


def kernel(**inputs):
    in_maps, so_full = _make_in_maps(**inputs)
    nc = _get_nc()
    res = run_bass_kernel_spmd(
        nc, in_maps, core_ids=list(range(N_CORES)), trace=TRACE,
        trace_cores=list(range(N_CORES)) if TRACE and TRACE_ALL_CORES else None,
    )
    if TRACE:
        kernel.last_exec_time_ns = res.exec_time_ns
        kernel.last_mean_exec_time_ns = res.mean_exec_time_ns

    y = np.zeros((T, HIDDEN), np.float32)
    for i in range(N_CORES):
        y += res.results[i]["y"].astype(np.float32)
    y *= so_full[None, :]
    return y.reshape(1, T, HIDDEN)
